# revision 18
# baseline (speedup 1.0000x reference)
"""Multi-head graph attention layer (GAT) for Trainium2, 8-core data-parallel.

Problem: B=8, N=1024, D_IN=256, D_OUT=64, H=8, LeakyReLU slope 0.2.
  Wh = einsum('bnd,hdf->bhnf', h, W)
  f1 = Wh @ a1, f2 = Wh @ a2              (per head)
  e  = leaky_relu(f1[:,None] + f2[None,:])
  att = softmax(where(adj==0, -inf, e))
  out = att @ Wh  -> concat heads [B, N, H*F]

Sharding: one batch element per NeuronCore (B=8 across 8 cores).

Algebra: with x = f1_i + f2_j,
  exp(leaky_relu(x)) = max(exp(x), exp(0.2 x))
                     = E1s_i * max(R_i * E2_j, E2s_j)
where R = exp(0.8 f1), E2 = exp(f2), E2s = exp(0.2 f2), E1s = exp(0.2 f1).
The E1s_i factor is constant along j, so it cancels between the softmax
numerator and denominator and is never computed.  Per (head, j-tile) the
[128, N] unnormalized attention U' = max(R_i*E2_j, E2s_j) * adj_ij needs
only TWO elementwise ops:
  1. a dual-op tensor_scalar (mult by per-partition E2_j, max with
     per-partition E2s_j) streaming the broadcast R row — 4x DVE mode;
  2. a tensor_tensor multiply with the adjacency tile (2x DVE mode,
     some tiles offloaded to GPSIMD).
U' accumulates against [Wh | 1] so column 64 of out^T is the softmax
denominator Z; normalization happens after a PE transpose.
"""

import numpy as np
import ml_dtypes

BF16 = ml_dtypes.bfloat16

B, N, D_IN, D_OUT, H = 8, 1024, 256, 64, 8
NEG_SLOPE = 0.2
P = 128                       # partitions
NJT = N // P                  # 8 j-tiles
NIT = N // P                  # 8 i-tiles
NKT = D_IN // P               # 2 k-tiles
HF = H * D_OUT                # 512
AUG = D_OUT + 1               # 65 (Wh columns + ones column)
FT_W = 40                     # f-score block width: f1 cols 0:8, f2 cols 32:40

# GPSIMD is kept OFF the [N,N] elementwise path: concurrent Pool streaming
# steals SBUF ports and collapses DVE's 4x tensor_scalar mode to 1x
# (measured 443ns -> 2260ns on every overlapped instance).


def _build_program():
    """Build the single-core SPMD Bass program. Returns nc."""
    import concourse.bass as bass
    import concourse.bacc as bacc
    import concourse.tile as tile
    from concourse import mybir
    from concourse.masks import make_identity

    f32 = mybir.dt.float32
    bf16 = mybir.dt.bfloat16
    AF = mybir.ActivationFunctionType
    OP = mybir.AluOpType

    nc = bacc.Bacc("TRN2", target_bir_lowering=False, debug=False,
                   enable_asserts=False, num_devices=8)

    hT = nc.dram_tensor("hT", [D_IN, N], bf16, kind="ExternalInput").ap()
    adjT = nc.dram_tensor("adjT", [N, N], bf16, kind="ExternalInput").ap()
    wrs = nc.dram_tensor("wrs", [D_IN, HF], bf16, kind="ExternalInput").ap()
    w12 = nc.dram_tensor("w12", [D_IN, FT_W], bf16,
                         kind="ExternalInput").ap()
    out = nc.dram_tensor("out", [N, HF], bf16, kind="ExternalOutput").ap()

    with tile.TileContext(nc) as tc:
        with (
            tc.tile_pool(name="const", bufs=1) as const,
            tc.tile_pool(name="inputs", bufs=1) as inputs,
            tc.tile_pool(name="whp", bufs=1) as whp,
            tc.tile_pool(name="ecol", bufs=1) as ecolp,
            tc.tile_pool(name="ps_s", bufs=2, space="PSUM") as ps_s,
            tc.tile_pool(name="ps_ot", bufs=2, space="PSUM") as ps_ot,
            tc.tile_pool(name="ps_tr", bufs=1, space="PSUM") as ps_tr,
            tc.tile_pool(name="work", bufs=3) as work,
            tc.tile_pool(name="fin", bufs=3) as fin,
            tc.tile_pool(name="dram", bufs=1, space="DRAM") as dramp,
        ):
            # ---- Phase 0: load inputs -------------------------------------
            # DMA order follows first-use: hT/w12 feed the f-score chain
            # that gates the R broadcast (the critical prolog path).
            ident = const.tile([P, P], f32)
            make_identity(nc, ident)

            ht_sb = []
            for kt in range(NKT):
                t = inputs.tile([P, N], bf16, tag=f"ht{kt}")
                for c in range(4):      # 4 column-chunk DMAs -> 4 queues
                    nc.sync.dma_start(
                        out=t[:, c * 256:(c + 1) * 256],
                        in_=hT[kt * P:(kt + 1) * P, c * 256:(c + 1) * 256])
                ht_sb.append(t)
            w12_sb = []
            for kt in range(NKT):
                t = inputs.tile([P, FT_W], bf16, tag=f"w12{kt}")
                nc.sync.dma_start(out=t, in_=w12[kt * P:(kt + 1) * P, :])
                w12_sb.append(t)

            # ---- Phase 1: f scores (transposed), exps, R broadcast --------
            # fT[0:8, :]  = f1 per head (transposed);  fT[8:16, :] = f2.
            # e12t[h, :]  = R   = exp(0.8 f1_h)   (broadcast row)
            # e2sb[h, :]  = E2  = exp(f2_h); e2sb[8+h, :] = E2s = exp(.2 f2_h)
            ft = ps_tr.tile([FT_W, N], f32, tag='big')
            for nh in range(2):
                for kt in range(NKT):
                    nc.tensor.matmul(ft[:, nh * 512:(nh + 1) * 512],
                                     w12_sb[kt],
                                     ht_sb[kt][:, nh * 512:(nh + 1) * 512],
                                     start=(kt == 0), stop=(kt == NKT - 1))
            e12t = const.tile([H, N], bf16)
            nc.scalar.activation(e12t, ft[0:H, :], AF.Exp,
                                 scale=1.0 - NEG_SLOPE)
            e2sb = const.tile([FT_W, N], f32)
            nc.vector.memset(e2sb, 0.0)
            nc.scalar.activation(e2sb[0:H, :], ft[32:32 + H, :], AF.Exp,
                                 scale=1.0)
            nc.scalar.activation(e2sb[32:32 + H, :], ft[32:32 + H, :],
                                 AF.Exp, scale=NEG_SLOPE)

            # bounce e12t through DRAM; broadcast all 8 head-rows up front
            e12t_dram = dramp.tile([H, N], bf16)
            nc.sync.dma_start(out=e12t_dram, in_=e12t)
            rall = const.tile([P, H, N], bf16)
            for h in range(H):
                for c in range(2):      # split across partitions -> 2 queues
                    nc.sync.dma_start(
                        out=rall[c * 64:(c + 1) * 64, h, :],
                        in_=e12t_dram[h:h + 1, :].partition_broadcast(64))

            # ecols[jt][:, 0:8] = E2_j, [:, 8:16] = E2s_j (per-partition)
            ecols = []
            for jt in range(NJT):
                trc = ps_s.tile([P, FT_W], f32, tag='pss')
                nc.tensor.transpose(trc, e2sb[:, jt * P:(jt + 1) * P],
                                    ident[0:FT_W, 0:FT_W])
                ec = ecolp.tile([P, FT_W], f32, tag=f"ecols{jt}")
                nc.scalar.copy(ec, trc)
                ecols.append(ec)

            # remaining inputs (first needed after the f-score chain)
            wrs_sb = []
            for kt in range(NKT):
                t = inputs.tile([P, HF], bf16, tag=f"wrs{kt}")
                nc.sync.dma_start(out=t, in_=wrs[kt * P:(kt + 1) * P, :])
                wrs_sb.append(t)
            adj_sb = []
            for jt in range(NJT):
                t = inputs.tile([P, N], bf16, tag=f"adj{jt}")
                nc.sync.dma_start(out=t, in_=adjT[jt * P:(jt + 1) * P, :])
                adj_sb.append(t)

            # ---- Phase 2: whaug = [Wh | 1] (no per-head scaling) ----------
            whaug = []
            for jt in range(NJT):
                ps = ps_s.tile([P, HF], f32, tag='pss')
                for kt in range(NKT):
                    lhsT = ht_sb[kt][:, jt * P:(jt + 1) * P]
                    nc.tensor.matmul(ps, lhsT, wrs_sb[kt],
                                     start=(kt == 0), stop=(kt == NKT - 1))
                wa = whp.tile([P, H, AUG], bf16, tag=f"whaug{jt}")
                nc.scalar.activation(wa[:, :, 0:D_OUT], ps, AF.Copy,
                                     scale=1.0)
                nc.vector.memset(wa[:, :, D_OUT], 1.0)
                whaug.append(wa)

            # output accumulators: [128, 512] bf16 per i-tile
            out_sb = []
            for it in range(NIT):
                osb = whp.tile([P, HF], bf16, tag=f"osb{it}")
                out_sb.append(osb)

            # ---- Phase 3: per-head-pair attention -------------------------
            def finalize(h, ot, muls_on_dve=False):
                ots = fin.tile([AUG, N], f32, tag="ots")
                nc.scalar.copy(ots, ot)
                # [P, NIT, P] so each transpose's 65-col window stays in one
                # PSUM bank (stride 128; only cols 0:65 of each slot used)
                tr2all = ps_tr.tile([P, NIT, P], f32, tag='big')
                for it in range(NIT):
                    nc.tensor.transpose(tr2all[:, it, 0:AUG],
                                        ots[:, it * P:(it + 1) * P],
                                        ident[0:AUG, 0:AUG])
                rcall = fin.tile([P, NIT], f32, tag="rcall")
                nc.vector.reciprocal(rcall, tr2all[:, :, D_OUT])
                for it in range(NIT):
                    dst = out_sb[it][:, h * D_OUT:(h + 1) * D_OUT]
                    if muls_on_dve:
                        nc.vector.tensor_scalar_mul(
                            dst, tr2all[:, it, 0:D_OUT], rcall[:, it:it + 1])
                    else:
                        nc.scalar.activation(dst, tr2all[:, it, 0:D_OUT],
                                             AF.Copy,
                                             scale=rcall[:, it:it + 1])

            for hp in range(H // 2):
                h0, h1 = 2 * hp, 2 * hp + 1
                ot0 = ps_ot.tile([AUG, N], f32, tag="ot")
                ot1 = ps_ot.tile([AUG, N], f32, tag="ot")
                for jt in range(NJT):
                    ec = ecols[jt]
                    # pt = max(R_i * E2_j, E2s_j)  (dual-op ts, 4x DVE mode)
                    pt2 = work.tile([P, 2, N], bf16, tag="pt", bufs=4)
                    for ph, h in ((0, h0), (1, h1)):
                        nc.vector.tensor_scalar(
                            out=pt2[:, ph, :], in0=rall[:, h, :],
                            scalar1=ec[:, h:h + 1],
                            scalar2=ec[:, 32 + h:32 + h + 1],
                            op0=OP.mult, op1=OP.max)
                    # um = pt * adj   (both heads in one 2x DVE pass)
                    um2 = work.tile([P, 2, N], bf16, tag="um", bufs=8)
                    nc.vector.tensor_tensor(
                        out=um2, in0=pt2,
                        in1=adj_sb[jt].unsqueeze(1).broadcast_to([P, 2, N]),
                        op=OP.mult)
                    # out^T[(f|1), i] += [Wh | 1]^T @ um
                    for ph, h, ot in ((0, h0, ot0), (1, h1, ot1)):
                        lhsT = whaug[jt][:, h, :]
                        for nh in range(2):
                            nc.tensor.matmul(
                                ot[:, nh * 512:(nh + 1) * 512], lhsT,
                                um2[:, ph, nh * 512:(nh + 1) * 512],
                                start=(jt == 0), stop=(jt == NJT - 1))
                last = hp == H // 2 - 1
                finalize(h0, ot0, muls_on_dve=last)
                finalize(h1, ot1, muls_on_dve=last)

            for it in range(NIT):
                nc.sync.dma_start(out=out[it * P:(it + 1) * P, :],
                                  in_=out_sb[it])

    nc.compile()
    return nc


def _host_prep(h, adj, W, a):
    """Host-side input prep: transposes / casts / tiny einsums only."""
    a1, a2 = a[:, :D_OUT], a[:, D_OUT:]
    w1 = np.einsum("hdf,hf->hd", W, a1).astype(np.float32)   # [H, D_IN]
    w2 = np.einsum("hdf,hf->hd", W, a2).astype(np.float32)
    w12 = np.zeros((D_IN, FT_W), dtype=np.float32)           # [D_IN, 40]
    w12[:, 0:H] = w1.T
    w12[:, 32:32 + H] = w2.T
    w12 = w12.astype(BF16)
    wrs = np.ascontiguousarray(
        W.transpose(1, 0, 2).reshape(D_IN, HF)).astype(BF16)
    in_maps = []
    for b in range(B):
        in_maps.append({
            "hT": np.ascontiguousarray(h[b].T).astype(BF16),
            "adjT": np.ascontiguousarray(adj[b].T).astype(BF16),
            "wrs": wrs,
            "w12": w12,
        })
    return in_maps


def kernel(h, adj, W, a):
    from concourse.bass_utils import run_bass_kernel_spmd

    in_maps = _host_prep(np.asarray(h), np.asarray(adj),
                         np.asarray(W), np.asarray(a))
    nc = _build_program()
    res = run_bass_kernel_spmd(nc, in_maps, core_ids=list(range(B)))
    out = np.stack([np.asarray(res.results[b]["out"]) for b in range(B)])
    return out.astype(np.float32)


# revision 19
# speedup vs baseline: 1.0319x; 1.0319x over previous
"""Multi-head graph attention layer (GAT) for Trainium2, 8-core data-parallel.

Problem: B=8, N=1024, D_IN=256, D_OUT=64, H=8, LeakyReLU slope 0.2.
  Wh = einsum('bnd,hdf->bhnf', h, W)
  f1 = Wh @ a1, f2 = Wh @ a2              (per head)
  e  = leaky_relu(f1[:,None] + f2[None,:])
  att = softmax(where(adj==0, -inf, e))
  out = att @ Wh  -> concat heads [B, N, H*F]

Sharding: one batch element per NeuronCore (B=8 across 8 cores).

Algebra: with x = f1_i + f2_j,
  exp(leaky_relu(x)) = max(exp(x), exp(0.2 x))
                     = E1s_i * max(R_i * E2_j, E2s_j)
where R = exp(0.8 f1), E2 = exp(f2), E2s = exp(0.2 f2), E1s = exp(0.2 f1).
The E1s_i factor is constant along j, so it cancels between the softmax
numerator and denominator and is never computed.  Per (head, j-tile) the
[128, N] unnormalized attention U' = max(R_i*E2_j, E2s_j) * adj_ij needs
only TWO elementwise ops:
  1. a dual-op tensor_scalar (mult by per-partition E2_j, max with
     per-partition E2s_j) streaming the broadcast R row — 4x DVE mode;
  2. a tensor_tensor multiply with the adjacency tile (2x DVE mode,
     some tiles offloaded to GPSIMD).
U' accumulates against [Wh | 1] so column 64 of out^T is the softmax
denominator Z; normalization happens after a PE transpose.
"""

import numpy as np
import ml_dtypes

BF16 = ml_dtypes.bfloat16

B, N, D_IN, D_OUT, H = 8, 1024, 256, 64, 8
NEG_SLOPE = 0.2
P = 128                       # partitions
NJT = N // P                  # 8 j-tiles
NIT = N // P                  # 8 i-tiles
NKT = D_IN // P               # 2 k-tiles
HF = H * D_OUT                # 512
AUG = D_OUT + 1               # 65 (Wh columns + ones column)
FT_W = 40                     # f-score block width: f1 cols 0:8, f2 cols 32:40

# GPSIMD is kept OFF the [N,N] elementwise path: concurrent Pool streaming
# steals SBUF ports and collapses DVE's 4x tensor_scalar mode to 1x
# (measured 443ns -> 2260ns on every overlapped instance).


def _build_program():
    """Build the single-core SPMD Bass program. Returns nc."""
    import concourse.bass as bass
    import concourse.bacc as bacc
    import concourse.tile as tile
    from concourse import mybir
    from concourse.masks import make_identity

    f32 = mybir.dt.float32
    bf16 = mybir.dt.bfloat16
    AF = mybir.ActivationFunctionType
    OP = mybir.AluOpType

    nc = bacc.Bacc("TRN2", target_bir_lowering=False, debug=False,
                   enable_asserts=False, num_devices=8)

    hT = nc.dram_tensor("hT", [D_IN, N], bf16, kind="ExternalInput").ap()
    adjT = nc.dram_tensor("adjT", [N, N], bf16, kind="ExternalInput").ap()
    wrs = nc.dram_tensor("wrs", [D_IN, HF], bf16, kind="ExternalInput").ap()
    w12 = nc.dram_tensor("w12", [D_IN, FT_W], bf16,
                         kind="ExternalInput").ap()
    out = nc.dram_tensor("out", [N, HF], bf16, kind="ExternalOutput").ap()

    with tile.TileContext(nc) as tc:
        with (
            tc.tile_pool(name="const", bufs=1) as const,
            tc.tile_pool(name="inputs", bufs=1) as inputs,
            tc.tile_pool(name="whp", bufs=1) as whp,
            tc.tile_pool(name="ecol", bufs=1) as ecolp,
            tc.tile_pool(name="ps_s", bufs=2, space="PSUM") as ps_s,
            tc.tile_pool(name="ps_ot", bufs=2, space="PSUM") as ps_ot,
            tc.tile_pool(name="ps_tr", bufs=1, space="PSUM") as ps_tr,
            tc.tile_pool(name="work", bufs=3) as work,
            tc.tile_pool(name="fin", bufs=3) as fin,
            tc.tile_pool(name="dram", bufs=1, space="DRAM") as dramp,
        ):
            # ---- Phase 0: load inputs -------------------------------------
            # DMA order follows first-use: hT/w12 feed the f-score chain
            # that gates the R broadcast (the critical prolog path).
            ident = const.tile([P, P], f32)
            make_identity(nc, ident)

            ht_sb = []
            for kt in range(NKT):
                t = inputs.tile([P, N], bf16, tag=f"ht{kt}")
                nc.gpsimd.dma_start(out=t, in_=hT[kt * P:(kt + 1) * P, :])
                ht_sb.append(t)
            w12_sb = []
            for kt in range(NKT):
                t = inputs.tile([P, FT_W], bf16, tag=f"w12{kt}")
                nc.gpsimd.dma_start(out=t, in_=w12[kt * P:(kt + 1) * P, :])
                w12_sb.append(t)

            # ---- Phase 1: f scores (transposed), exps, R broadcast --------
            # fT[0:8, :]  = f1 per head (transposed);  fT[8:16, :] = f2.
            # e12t[h, :]  = R   = exp(0.8 f1_h)   (broadcast row)
            # e2sb[h, :]  = E2  = exp(f2_h); e2sb[8+h, :] = E2s = exp(.2 f2_h)
            ft = ps_tr.tile([FT_W, N], f32, tag='big')
            for nh in range(2):
                for kt in range(NKT):
                    nc.tensor.matmul(ft[:, nh * 512:(nh + 1) * 512],
                                     w12_sb[kt],
                                     ht_sb[kt][:, nh * 512:(nh + 1) * 512],
                                     start=(kt == 0), stop=(kt == NKT - 1))
            e12t = const.tile([H, N], bf16)
            nc.scalar.activation(e12t, ft[0:H, :], AF.Exp,
                                 scale=1.0 - NEG_SLOPE)
            e2sb = const.tile([FT_W, N], f32)
            nc.vector.memset(e2sb, 0.0)
            nc.scalar.activation(e2sb[0:H, :], ft[32:32 + H, :], AF.Exp,
                                 scale=1.0)
            nc.scalar.activation(e2sb[32:32 + H, :], ft[32:32 + H, :],
                                 AF.Exp, scale=NEG_SLOPE)

            # bounce e12t through DRAM; broadcast all 8 head-rows up front
            e12t_dram = dramp.tile([H, N], bf16)
            nc.gpsimd.dma_start(out=e12t_dram, in_=e12t)
            rall = const.tile([P, H, N], bf16)
            for h in range(H):
                nc.gpsimd.dma_start(
                    out=rall[:, h, :],
                    in_=e12t_dram[h:h + 1, :].partition_broadcast(P))

            # ecols[jt][:, 0:8] = E2_j, [:, 8:16] = E2s_j (per-partition)
            ecols = []
            for jt in range(NJT):
                trc = ps_s.tile([P, FT_W], f32, tag='pss')
                nc.tensor.transpose(trc, e2sb[:, jt * P:(jt + 1) * P],
                                    ident[0:FT_W, 0:FT_W])
                ec = ecolp.tile([P, FT_W], f32, tag=f"ecols{jt}")
                nc.scalar.copy(ec, trc)
                ecols.append(ec)

            # remaining inputs (first needed after the f-score chain)
            wrs_sb = []
            for kt in range(NKT):
                t = inputs.tile([P, HF], bf16, tag=f"wrs{kt}")
                nc.gpsimd.dma_start(out=t, in_=wrs[kt * P:(kt + 1) * P, :])
                wrs_sb.append(t)
            adj_sb = []
            for jt in range(NJT):
                t = inputs.tile([P, N], bf16, tag=f"adj{jt}")
                nc.gpsimd.dma_start(out=t, in_=adjT[jt * P:(jt + 1) * P, :])
                adj_sb.append(t)

            # ---- Phase 2: whaug = [Wh | 1] (no per-head scaling) ----------
            whaug = []
            for jt in range(NJT):
                ps = ps_s.tile([P, HF], f32, tag='pss')
                for kt in range(NKT):
                    lhsT = ht_sb[kt][:, jt * P:(jt + 1) * P]
                    nc.tensor.matmul(ps, lhsT, wrs_sb[kt],
                                     start=(kt == 0), stop=(kt == NKT - 1))
                wa = whp.tile([P, H, AUG], bf16, tag=f"whaug{jt}")
                nc.scalar.activation(wa[:, :, 0:D_OUT], ps, AF.Copy,
                                     scale=1.0)
                nc.vector.memset(wa[:, :, D_OUT], 1.0)
                whaug.append(wa)

            # output accumulators: [128, 512] bf16 per i-tile
            out_sb = []
            for it in range(NIT):
                osb = whp.tile([P, HF], bf16, tag=f"osb{it}")
                out_sb.append(osb)

            # ---- Phase 3: per-head-pair attention -------------------------
            def finalize(h, ot, muls_on_dve=False):
                ots = fin.tile([AUG, N], f32, tag="ots")
                nc.scalar.copy(ots, ot)
                # [P, NIT, P] so each transpose's 65-col window stays in one
                # PSUM bank (stride 128; only cols 0:65 of each slot used)
                tr2all = ps_tr.tile([P, NIT, P], f32, tag='big')
                for it in range(NIT):
                    nc.tensor.transpose(tr2all[:, it, 0:AUG],
                                        ots[:, it * P:(it + 1) * P],
                                        ident[0:AUG, 0:AUG])
                rcall = fin.tile([P, NIT], f32, tag="rcall")
                nc.vector.reciprocal(rcall, tr2all[:, :, D_OUT])
                for it in range(NIT):
                    dst = out_sb[it][:, h * D_OUT:(h + 1) * D_OUT]
                    if muls_on_dve:
                        nc.vector.tensor_scalar_mul(
                            dst, tr2all[:, it, 0:D_OUT], rcall[:, it:it + 1])
                    else:
                        nc.scalar.activation(dst, tr2all[:, it, 0:D_OUT],
                                             AF.Copy,
                                             scale=rcall[:, it:it + 1])

            for hp in range(H // 2):
                h0, h1 = 2 * hp, 2 * hp + 1
                ot0 = ps_ot.tile([AUG, N], f32, tag="ot")
                ot1 = ps_ot.tile([AUG, N], f32, tag="ot")
                for jt in range(NJT):
                    ec = ecols[jt]
                    # pt = max(R_i * E2_j, E2s_j)  (dual-op ts, 4x DVE mode)
                    pt2 = work.tile([P, 2, N], bf16, tag="pt", bufs=4)
                    for ph, h in ((0, h0), (1, h1)):
                        nc.vector.tensor_scalar(
                            out=pt2[:, ph, :], in0=rall[:, h, :],
                            scalar1=ec[:, h:h + 1],
                            scalar2=ec[:, 32 + h:32 + h + 1],
                            op0=OP.mult, op1=OP.max)
                    # um = pt * adj   (both heads in one 2x DVE pass)
                    um2 = work.tile([P, 2, N], bf16, tag="um", bufs=8)
                    nc.vector.tensor_tensor(
                        out=um2, in0=pt2,
                        in1=adj_sb[jt].unsqueeze(1).broadcast_to([P, 2, N]),
                        op=OP.mult)
                    # out^T[(f|1), i] += [Wh | 1]^T @ um
                    for ph, h, ot in ((0, h0, ot0), (1, h1, ot1)):
                        lhsT = whaug[jt][:, h, :]
                        for nh in range(2):
                            nc.tensor.matmul(
                                ot[:, nh * 512:(nh + 1) * 512], lhsT,
                                um2[:, ph, nh * 512:(nh + 1) * 512],
                                start=(jt == 0), stop=(jt == NJT - 1))
                last = hp == H // 2 - 1
                finalize(h0, ot0, muls_on_dve=last)
                finalize(h1, ot1, muls_on_dve=last)

            for half in range(2):
                cs, ce = half * 256, (half + 1) * 256
                for it in range(NIT):
                    nc.sync.dma_start(out=out[it * P:(it + 1) * P, cs:ce],
                                      in_=out_sb[it][:, cs:ce])

    nc.compile()
    return nc


def _host_prep(h, adj, W, a):
    """Host-side input prep: transposes / casts / tiny einsums only."""
    a1, a2 = a[:, :D_OUT], a[:, D_OUT:]
    w1 = np.einsum("hdf,hf->hd", W, a1).astype(np.float32)   # [H, D_IN]
    w2 = np.einsum("hdf,hf->hd", W, a2).astype(np.float32)
    w12 = np.zeros((D_IN, FT_W), dtype=np.float32)           # [D_IN, 40]
    w12[:, 0:H] = w1.T
    w12[:, 32:32 + H] = w2.T
    w12 = w12.astype(BF16)
    wrs = np.ascontiguousarray(
        W.transpose(1, 0, 2).reshape(D_IN, HF)).astype(BF16)
    in_maps = []
    for b in range(B):
        in_maps.append({
            "hT": np.ascontiguousarray(h[b].T).astype(BF16),
            "adjT": np.ascontiguousarray(adj[b].T).astype(BF16),
            "wrs": wrs,
            "w12": w12,
        })
    return in_maps


def kernel(h, adj, W, a):
    from concourse.bass_utils import run_bass_kernel_spmd

    in_maps = _host_prep(np.asarray(h), np.asarray(adj),
                         np.asarray(W), np.asarray(a))
    nc = _build_program()
    res = run_bass_kernel_spmd(nc, in_maps, core_ids=list(range(B)))
    out = np.stack([np.asarray(res.results[b]["out"]) for b in range(B)])
    return out.astype(np.float32)


# revision 21
# speedup vs baseline: 1.0443x; 1.0120x over previous
"""Multi-head graph attention layer (GAT) for Trainium2, 8-core data-parallel.

Problem: B=8, N=1024, D_IN=256, D_OUT=64, H=8, LeakyReLU slope 0.2.
  Wh = einsum('bnd,hdf->bhnf', h, W)
  f1 = Wh @ a1, f2 = Wh @ a2              (per head)
  e  = leaky_relu(f1[:,None] + f2[None,:])
  att = softmax(where(adj==0, -inf, e))
  out = att @ Wh  -> concat heads [B, N, H*F]

Sharding: one batch element per NeuronCore (B=8 across 8 cores).

Algebra: with x = f1_i + f2_j,
  exp(leaky_relu(x)) = max(exp(x), exp(0.2 x))
                     = E1s_i * max(R_i * E2_j, E2s_j)
where R = exp(0.8 f1), E2 = exp(f2), E2s = exp(0.2 f2), E1s = exp(0.2 f1).
The E1s_i factor is constant along j, so it cancels between the softmax
numerator and denominator and is never computed.  Per (head, j-tile) the
[128, N] unnormalized attention U' = max(R_i*E2_j, E2s_j) * adj_ij needs
only TWO elementwise ops:
  1. a dual-op tensor_scalar (mult by per-partition E2_j, max with
     per-partition E2s_j) streaming the broadcast R row — 4x DVE mode;
  2. a tensor_tensor multiply with the adjacency tile (2x DVE mode,
     some tiles offloaded to GPSIMD).
U' accumulates against [Wh | 1] so column 64 of out^T is the softmax
denominator Z; normalization happens after a PE transpose.
"""

import numpy as np
import ml_dtypes

BF16 = ml_dtypes.bfloat16

B, N, D_IN, D_OUT, H = 8, 1024, 256, 64, 8
NEG_SLOPE = 0.2
P = 128                       # partitions
NJT = N // P                  # 8 j-tiles
NIT = N // P                  # 8 i-tiles
NKT = D_IN // P               # 2 k-tiles
HF = H * D_OUT                # 512
AUG = D_OUT + 1               # 65 (Wh columns + ones column)
FT_W = 40                     # f-score block width: f1 cols 0:8, f2 cols 32:40

# GPSIMD is kept OFF the [N,N] elementwise path: concurrent Pool streaming
# steals SBUF ports and collapses DVE's 4x tensor_scalar mode to 1x
# (measured 443ns -> 2260ns on every overlapped instance).


def _build_program():
    """Build the single-core SPMD Bass program. Returns nc."""
    import concourse.bass as bass
    import concourse.bacc as bacc
    import concourse.tile as tile
    from concourse import mybir
    from concourse.masks import make_identity

    f32 = mybir.dt.float32
    bf16 = mybir.dt.bfloat16
    AF = mybir.ActivationFunctionType
    OP = mybir.AluOpType

    nc = bacc.Bacc("TRN2", target_bir_lowering=False, debug=False,
                   enable_asserts=False, num_devices=8)

    hT = nc.dram_tensor("hT", [D_IN, N], bf16, kind="ExternalInput").ap()
    adjT = nc.dram_tensor("adjT", [N, N], bf16, kind="ExternalInput").ap()
    wrs = nc.dram_tensor("wrs", [D_IN, HF], bf16, kind="ExternalInput").ap()
    w12 = nc.dram_tensor("w12", [D_IN, FT_W], bf16,
                         kind="ExternalInput").ap()
    out = nc.dram_tensor("out", [N, HF], bf16, kind="ExternalOutput").ap()

    with tile.TileContext(nc) as tc:
        with (
            tc.tile_pool(name="const", bufs=1) as const,
            tc.tile_pool(name="inputs", bufs=1) as inputs,
            tc.tile_pool(name="whp", bufs=1) as whp,
            tc.tile_pool(name="ecol", bufs=1) as ecolp,
            tc.tile_pool(name="ps_s", bufs=2, space="PSUM") as ps_s,
            tc.tile_pool(name="ps_ot", bufs=2, space="PSUM") as ps_ot,
            tc.tile_pool(name="ps_tr", bufs=1, space="PSUM") as ps_tr,
            tc.tile_pool(name="work", bufs=3) as work,
            tc.tile_pool(name="fin", bufs=3) as fin,
            tc.tile_pool(name="dram", bufs=1, space="DRAM") as dramp,
        ):
            # ---- Phase 0: load inputs -------------------------------------
            # DMA order follows first-use: hT/w12 feed the f-score chain
            # that gates the R broadcast (the critical prolog path).
            ident = const.tile([P, P], f32)
            make_identity(nc, ident)

            ht_sb = []
            for kt in range(NKT):
                t = inputs.tile([P, N], bf16, tag=f"ht{kt}")
                nc.gpsimd.dma_start(out=t, in_=hT[kt * P:(kt + 1) * P, :])
                ht_sb.append(t)
            w12_sb = []
            for kt in range(NKT):
                t = inputs.tile([P, FT_W], bf16, tag=f"w12{kt}")
                nc.gpsimd.dma_start(out=t, in_=w12[kt * P:(kt + 1) * P, :])
                w12_sb.append(t)

            # ---- Phase 1: f scores, exps, R broadcast ---------------------
            # ecols[jt][:, h] = E2_j = exp(f2_h,j); [:, 32+h] = E2s_j
            # (direct [j-partition, 40] matmuls; independent of ft chain)
            ecols = []
            for jt in range(NJT):
                psec = ps_s.tile([P, FT_W], f32, tag='pss')
                for kt in range(NKT):
                    nc.tensor.matmul(psec, ht_sb[kt][:, jt * P:(jt + 1) * P],
                                     w12_sb[kt],
                                     start=(kt == 0), stop=(kt == NKT - 1))
                ec = ecolp.tile([P, FT_W], f32, tag=f"ecols{jt}")
                nc.scalar.activation(ec[:, 0:H], psec[:, 32:32 + H], AF.Exp,
                                     scale=1.0)
                nc.scalar.activation(ec[:, 32:32 + H], psec[:, 32:32 + H],
                                     AF.Exp, scale=NEG_SLOPE)
                ecols.append(ec)

            # ft[h, :] = f1_h (transposed);  e12t = exp(0.8 f1) -> broadcast
            ft = ps_tr.tile([H, N], f32, tag='big')
            for nh in range(2):
                for kt in range(NKT):
                    nc.tensor.matmul(ft[:, nh * 512:(nh + 1) * 512],
                                     w12_sb[kt][:, 0:H],
                                     ht_sb[kt][:, nh * 512:(nh + 1) * 512],
                                     start=(kt == 0), stop=(kt == NKT - 1))
            e12t = const.tile([H, N], bf16)
            nc.scalar.activation(e12t, ft, AF.Exp, scale=1.0 - NEG_SLOPE)

            # bounce e12t through DRAM; per-head broadcast tiles so head h's
            # first fused op waits only on its own broadcast
            e12t_dram = dramp.tile([H, N], bf16)
            nc.gpsimd.dma_start(out=e12t_dram, in_=e12t)
            rall = []
            for h in range(H):
                rt = const.tile([P, N], bf16, tag=f"rall{h}")
                nc.gpsimd.dma_start(
                    out=rt,
                    in_=e12t_dram[h:h + 1, :].partition_broadcast(P))
                rall.append(rt)

            # remaining inputs (first needed after the f-score chain)
            wrs_sb = []
            for kt in range(NKT):
                t = inputs.tile([P, HF], bf16, tag=f"wrs{kt}")
                nc.gpsimd.dma_start(out=t, in_=wrs[kt * P:(kt + 1) * P, :])
                wrs_sb.append(t)
            adj_sb = []
            for jt in range(NJT):
                t = inputs.tile([P, N], bf16, tag=f"adj{jt}")
                nc.gpsimd.dma_start(out=t, in_=adjT[jt * P:(jt + 1) * P, :])
                adj_sb.append(t)

            # ---- Phase 2: whaug = [Wh | 1] (no per-head scaling) ----------
            whaug = []
            for jt in range(NJT):
                ps = ps_s.tile([P, HF], f32, tag='pss')
                for kt in range(NKT):
                    lhsT = ht_sb[kt][:, jt * P:(jt + 1) * P]
                    nc.tensor.matmul(ps, lhsT, wrs_sb[kt],
                                     start=(kt == 0), stop=(kt == NKT - 1))
                wa = whp.tile([P, H, AUG], bf16, tag=f"whaug{jt}")
                nc.scalar.activation(wa[:, :, 0:D_OUT], ps, AF.Copy,
                                     scale=1.0)
                nc.vector.memset(wa[:, :, D_OUT], 1.0)
                whaug.append(wa)

            # output accumulators, split in column halves per i-tile so the
            # heads 0-3 half can DMA out mid-run (subtile deps are whole-tile)
            out_lo, out_hi = [], []
            for it in range(NIT):
                osl = whp.tile([P, HF // 2], bf16, tag=f"osbl{it}")
                osh = whp.tile([P, HF // 2], bf16, tag=f"osbh{it}")
                out_lo.append(osl)
                out_hi.append(osh)

            # ---- Phase 3: per-head-pair attention -------------------------
            def finalize(h, ot, muls_on_dve=False):
                ots = fin.tile([AUG, N], f32, tag="ots")
                nc.scalar.copy(ots, ot)
                # [P, NIT, P] so each transpose's 65-col window stays in one
                # PSUM bank (stride 128; only cols 0:65 of each slot used)
                tr2all = ps_tr.tile([P, NIT, P], f32, tag='big')
                for it in range(NIT):
                    nc.tensor.transpose(tr2all[:, it, 0:AUG],
                                        ots[:, it * P:(it + 1) * P],
                                        ident[0:AUG, 0:AUG])
                rcall = fin.tile([P, NIT], f32, tag="rcall")
                nc.vector.reciprocal(rcall, tr2all[:, :, D_OUT])
                osb, hh = (out_lo, h) if h < 4 else (out_hi, h - 4)
                for it in range(NIT):
                    dst = osb[it][:, hh * D_OUT:(hh + 1) * D_OUT]
                    if muls_on_dve:
                        nc.vector.tensor_scalar_mul(
                            dst, tr2all[:, it, 0:D_OUT], rcall[:, it:it + 1])
                    else:
                        nc.scalar.activation(dst, tr2all[:, it, 0:D_OUT],
                                             AF.Copy,
                                             scale=rcall[:, it:it + 1])

            for hp in range(H // 2):
                h0, h1 = 2 * hp, 2 * hp + 1
                ot0 = ps_ot.tile([AUG, N], f32, tag="ot")
                ot1 = ps_ot.tile([AUG, N], f32, tag="ot")
                for jt in range(NJT):
                    ec = ecols[jt]
                    # pt = max(R_i * E2_j, E2s_j)  (dual-op ts, 4x DVE mode)
                    pt2 = work.tile([P, 2, N], bf16, tag="pt", bufs=4)
                    for ph, h in ((0, h0), (1, h1)):
                        nc.vector.tensor_scalar(
                            out=pt2[:, ph, :], in0=rall[h],
                            scalar1=ec[:, h:h + 1],
                            scalar2=ec[:, 32 + h:32 + h + 1],
                            op0=OP.mult, op1=OP.max)
                    # um = pt * adj   (both heads in one 2x DVE pass)
                    um2 = work.tile([P, 2, N], bf16, tag="um", bufs=8)
                    nc.vector.tensor_tensor(
                        out=um2, in0=pt2,
                        in1=adj_sb[jt].unsqueeze(1).broadcast_to([P, 2, N]),
                        op=OP.mult)
                    # out^T[(f|1), i] += [Wh | 1]^T @ um
                    for ph, h, ot in ((0, h0, ot0), (1, h1, ot1)):
                        lhsT = whaug[jt][:, h, :]
                        for nh in range(2):
                            nc.tensor.matmul(
                                ot[:, nh * 512:(nh + 1) * 512], lhsT,
                                um2[:, ph, nh * 512:(nh + 1) * 512],
                                start=(jt == 0), stop=(jt == NJT - 1))
                last = hp == H // 2 - 1
                finalize(h0, ot0, muls_on_dve=last)
                finalize(h1, ot1, muls_on_dve=last)

            for half, osb in ((0, out_lo), (1, out_hi)):
                cs = half * 256
                for it in range(NIT):
                    nc.sync.dma_start(
                        out=out[it * P:(it + 1) * P, cs:cs + 256],
                        in_=osb[it])

    nc.compile()
    return nc


def _host_prep(h, adj, W, a):
    """Host-side input prep: transposes / casts / tiny einsums only."""
    a1, a2 = a[:, :D_OUT], a[:, D_OUT:]
    w1 = np.einsum("hdf,hf->hd", W, a1).astype(np.float32)   # [H, D_IN]
    w2 = np.einsum("hdf,hf->hd", W, a2).astype(np.float32)
    w12 = np.zeros((D_IN, FT_W), dtype=np.float32)           # [D_IN, 40]
    w12[:, 0:H] = w1.T
    w12[:, 32:32 + H] = w2.T
    w12 = w12.astype(BF16)
    wrs = np.ascontiguousarray(
        W.transpose(1, 0, 2).reshape(D_IN, HF)).astype(BF16)
    in_maps = []
    for b in range(B):
        in_maps.append({
            "hT": np.ascontiguousarray(h[b].T).astype(BF16),
            "adjT": np.ascontiguousarray(adj[b].T).astype(BF16),
            "wrs": wrs,
            "w12": w12,
        })
    return in_maps


def kernel(h, adj, W, a):
    from concourse.bass_utils import run_bass_kernel_spmd

    in_maps = _host_prep(np.asarray(h), np.asarray(adj),
                         np.asarray(W), np.asarray(a))
    nc = _build_program()
    res = run_bass_kernel_spmd(nc, in_maps, core_ids=list(range(B)))
    out = np.stack([np.asarray(res.results[b]["out"]) for b in range(B)])
    return out.astype(np.float32)
